# revision 22
# baseline (speedup 1.0000x reference)
"""Trainium2 Bass kernel: batch-sharded fused attention (nn_Attention_48893907698232).

Reference computation (per batch b):
    q = x @ wq.T + bq ; k = x @ wk.T + bk ; v = x @ wv.T + bv
    q, k <- fixed RoPE rotation (same rotation at every position)
    out  = softmax((q @ k.T) / sqrt(512)) @ v

Strategy:
  * Data-parallel over batch: 16 batches -> 2 per NeuronCore, 8 cores, no
    collectives (attention is per-batch independent).
  * The RoPE rotation is position-independent here, so it is folded into the
    projection weights/biases on the host (w_rot = R @ w, b_rot = R @ b).
  * Host pre-transposes/pre-tiles x and weights into SBUF tile layout so every
    matmul contraction dim lands on SBUF partitions with zero on-chip
    transposes and every DMA row is contiguous:
      qT[o,s]  = sum_i wqT[i,o] * xT[i,s]     (PSUM out: o on partitions)
      kT[o,s]  likewise
      v[s,o]   = sum_i xT[i,s] * wvT[i,o]     (PSUM out: s on partitions)
      ST[sk,sq]= sum_d kT[d,sk] * qT[d,sq]    -> exp(scale*ST) on ACT -> ET
      out[sq,:]= sum_sk ET[sk,sq] * [v|1]     (ones column => softmax rowsum
                                               lands in output column 256)
      out  <- out[:, :256] * (1 / out[:, 256])
  * Softmax runs without max-subtraction: scores are ~N(0, 0.75) for this
    problem so exp() is far from overflow.
  * Matmul operands are bf16 (full-rate TensorEngine, fp32 PSUM accumulate);
    x and the folded weights are bf16-cast on the host, q/k/v/exp(S) are
    written as bf16 by their producing engines.
  * Input DMAs are split per 128-partition tile and triggered round-robin
    from the sync/gpsimd/vector/scalar queues so the first matmul group's
    operands land as early as possible.
"""

import math
import os
import sys

import numpy as np

os.environ.setdefault("MYCRO_LOCAL_CACHE", "1")
if "/opt/trn_rl_repo" not in sys.path:
    sys.path.insert(0, "/opt/trn_rl_repo")

from contextlib import ExitStack

import concourse.bass as bass
import concourse.tile as tile
from concourse import bacc, mybir
from concourse.bass_utils import run_bass_kernel_spmd

B, S, IN_DIM, OUT_DIM = 16, 1024, 512, 256
THETA = 10000.0
N_CORES = 8
B_LOC = B // N_CORES          # batches per core
I_T = IN_DIM // 128           # 4 contraction tiles for the projections
O_T = OUT_DIM // 128          # 2 feature tiles
S_TILES = S // 128            # 8 sequence tiles
SC = S // 512                 # 2 sequence chunks of 512
SCALE = 1.0 / math.sqrt(IN_DIM)

F32 = mybir.dt.float32
BF16 = mybir.dt.bfloat16


def _build():
    nc = bacc.Bacc(
        "TRN2",
        target_bir_lowering=False,
        debug=False,
        enable_asserts=False,
        num_devices=N_CORES,
    )
    # x pre-tiled on host: xh[b, h, p, i, s] = x[global_batch, h*512+s, i*128+p]
    # (p-major so each DMA partition row is I_T*512*2B = 4KB contiguous)
    xh = nc.dram_tensor(
        "xh", [B_LOC, SC, 128, I_T, 512], BF16, kind="ExternalInput"
    ).ap()
    # weights pre-tiled on host: w[p, i, o] = w_rot.T[i*128+p, o]
    wq = nc.dram_tensor("wq", [128, I_T, OUT_DIM], BF16, kind="ExternalInput").ap()
    wk = nc.dram_tensor("wk", [128, I_T, OUT_DIM], BF16, kind="ExternalInput").ap()
    wv = nc.dram_tensor("wv", [128, I_T, OUT_DIM], BF16, kind="ExternalInput").ap()
    bqr = nc.dram_tensor("bqr", [128, O_T], F32, kind="ExternalInput").ap()
    bkr = nc.dram_tensor("bkr", [128, O_T], F32, kind="ExternalInput").ap()
    bv = nc.dram_tensor("bv", [OUT_DIM], F32, kind="ExternalInput").ap()
    out = nc.dram_tensor("out", [B_LOC, S, OUT_DIM], F32, kind="ExternalOutput").ap()

    with tile.TileContext(nc) as tc, ExitStack() as ctx:
        sb = ctx.enter_context(tc.tile_pool(name="sb", bufs=1))
        small = ctx.enter_context(tc.tile_pool(name="small", bufs=4))
        ps_pj = ctx.enter_context(tc.tile_pool(name="ps_pj", bufs=3, space="PSUM"))
        ps_s = ctx.enter_context(tc.tile_pool(name="ps_s", bufs=3, space="PSUM"))
        ps_av = ctx.enter_context(tc.tile_pool(name="ps_av", bufs=2, space="PSUM"))

        # DMA trigger queues with explicit assignment: the first matmul's two
        # dependencies (x00i0, wq) go on DIFFERENT rings so its latency is
        # max(128KB, 256KB), not their sum. (Tensor queue stays clean.)
        S_, G_, C_ = nc.sync, nc.gpsimd, nc.scalar

        xsb = {}

        def load_xi(eng, b, h, i):
            t = sb.tile([128, 512], BF16, name=f"x{b}h{h}i{i}")
            eng.dma_start(out=t, in_=xh[b, h, :, i, :])
            xsb[b, h, i] = t

        load_xi(S_, 0, 0, 0)
        wq_sb = sb.tile([128, I_T, OUT_DIM], BF16, name="wq_sb")
        G_.dma_start(out=wq_sb, in_=wq)
        load_xi(C_, 0, 0, 1)
        load_xi(S_, 0, 0, 2)
        load_xi(C_, 0, 0, 3)
        wk_sb = sb.tile([128, I_T, OUT_DIM], BF16, name="wk_sb")
        C_.dma_start(out=wk_sb, in_=wk)
        wv_sb = sb.tile([128, I_T, OUT_DIM], BF16, name="wv_sb")
        S_.dma_start(out=wv_sb, in_=wv)
        bqr_sb = sb.tile([128, O_T], F32, name="bqr_sb")
        C_.dma_start(out=bqr_sb, in_=bqr)
        bkr_sb = sb.tile([128, O_T], F32, name="bkr_sb")
        C_.dma_start(out=bkr_sb, in_=bkr)
        bv_bc = sb.tile([128, OUT_DIM], F32, name="bv_bc")
        S_.dma_start(
            out=bv_bc,
            in_=bass.AP(tensor=bv.tensor, offset=bv.offset,
                        ap=[[0, 128], bv.ap[0]]))
        load_xi(G_, 0, 1, 0)
        load_xi(G_, 0, 1, 1)
        load_xi(S_, 0, 1, 2)
        load_xi(G_, 0, 1, 3)
        load_xi(C_, 1, 0, 0)
        load_xi(S_, 1, 0, 1)
        load_xi(G_, 1, 0, 2)
        load_xi(C_, 1, 0, 3)
        load_xi(S_, 1, 1, 0)
        load_xi(G_, 1, 1, 1)
        load_xi(C_, 1, 1, 2)
        load_xi(S_, 1, 1, 3)

        q_sb = [sb.tile([128, O_T, S], BF16, name=f"q{b}") for b in range(B_LOC)]
        k_sb = [sb.tile([128, O_T, S], BF16, name=f"k{b}") for b in range(B_LOC)]
        v_sb = [
            sb.tile([128, S_TILES, OUT_DIM + 1], BF16, name=f"v{b}")
            for b in range(B_LOC)
        ]
        e_sb = [sb.tile([128, S_TILES, S], BF16, name=f"e{b}") for b in range(B_LOC)]

        # ---- work-unit generators (each unit = one PSUM group + epilogue) ----
        def qkv_units(b):
            units = []
            for h in range(SC):
                for w_s, b_s, dst in (
                    (wq_sb, bqr_sb, q_sb[b]),
                    (wk_sb, bkr_sb, k_sb[b]),
                ):
                    for o in range(O_T):
                        def f(h=h, o=o, w_s=w_s, b_s=b_s, dst=dst):
                            ps = ps_pj.tile([128, 512], F32, tag="pj", name="pspj")
                            for i in range(I_T):
                                nc.tensor.matmul(
                                    ps,
                                    w_s[:, i, o * 128:(o + 1) * 128],
                                    xsb[b, h, i],
                                    start=(i == 0),
                                    stop=(i == I_T - 1),
                                )
                            nc.vector.tensor_scalar_add(
                                dst[:, o, h * 512:(h + 1) * 512], ps,
                                b_s[:, o:o + 1],
                            )
                        units.append(f)
                for j in range(S_TILES // SC):
                    def f(h=h, j=j):
                        st = h * (S_TILES // SC) + j
                        ps = ps_pj.tile([128, OUT_DIM], F32, tag="pj", name="pspj")
                        for i in range(I_T):
                            nc.tensor.matmul(
                                ps,
                                xsb[b, h, i][:, j * 128:(j + 1) * 128],
                                wv_sb[:, i, :],
                                start=(i == 0),
                                stop=(i == I_T - 1),
                            )
                        nc.vector.tensor_add(v_sb[b][:, st, 0:OUT_DIM], ps, bv_bc)
                        nc.vector.memset(v_sb[b][:, st, OUT_DIM:OUT_DIM + 1], 1.0)
                    units.append(f)
            return units

        def st_units(b):
            # scores^T tile (sk on partitions, sq chunk on free) + fused exp
            units = []
            for h in range(SC):
                for sk in range(S_TILES):
                    def f(h=h, sk=sk):
                        ps = ps_s.tile([128, 512], F32, tag="s", name="pss")
                        for d in range(O_T):
                            nc.tensor.matmul(
                                ps,
                                k_sb[b][:, d, sk * 128:(sk + 1) * 128],
                                q_sb[b][:, d, h * 512:(h + 1) * 512],
                                start=(d == 0),
                                stop=(d == O_T - 1),
                            )
                        nc.scalar.activation(
                            out=e_sb[b][:, sk, h * 512:(h + 1) * 512],
                            in_=ps,
                            func=mybir.ActivationFunctionType.Exp,
                            scale=SCALE,
                        )
                    units.append(f)
            return units

        def av_units(b, split_last=False):
            units = []
            for sq in range(S_TILES):
                def f(sq=sq):
                    ps = ps_av.tile([128, OUT_DIM + 1], F32, tag="av", name="psav")
                    for sk in range(S_TILES):
                        nc.tensor.matmul(
                            ps,
                            e_sb[b][:, sk, sq * 128:(sq + 1) * 128],
                            v_sb[b][:, sk, :],
                            start=(sk == 0),
                            stop=(sk == S_TILES - 1),
                        )
                    rec = small.tile([128, 1], F32, tag="rec", name="rec")
                    nc.vector.reciprocal(rec, ps[:, OUT_DIM:OUT_DIM + 1])
                    ot = small.tile([128, OUT_DIM], F32, tag="ot", name="ot")
                    if split_last and sq == S_TILES - 1:
                        half = OUT_DIM // 2
                        nc.vector.tensor_scalar_mul(ot[:, 0:half],
                                                    ps[:, 0:half], rec)
                        nc.sync.dma_start(
                            out=out[b, sq * 128:(sq + 1) * 128, 0:half],
                            in_=ot[:, 0:half],
                        )
                        nc.vector.tensor_scalar_mul(ot[:, half:OUT_DIM],
                                                    ps[:, half:OUT_DIM], rec)
                        nc.scalar.dma_start(
                            out=out[b, sq * 128:(sq + 1) * 128, half:OUT_DIM],
                            in_=ot[:, half:OUT_DIM],
                        )
                    else:
                        nc.vector.tensor_scalar_mul(ot, ps[:, 0:OUT_DIM], rec)
                        nc.sync.dma_start(
                            out=out[b, sq * 128:(sq + 1) * 128, :], in_=ot
                        )
                units.append(f)
            return units

        # ---- PE warm-up: keep TensorE busy while input DMAs land so the
        # HAM clock gate reaches 8/8 before the first real matmul ----
        junk = sb.tile([128, 128], BF16, name="junk")
        nc.vector.memset(junk, 0.0)
        for _ in range(8):
            wps = ps_pj.tile([128, 128], F32, tag="pj", name="warmps")
            for _k in range(4):
                nc.tensor.matmul(wps, junk, junk, start=(_k == 0),
                                 stop=(_k == 3))

        # ---- emission: interleave phases so PE never starves ----
        for u in qkv_units(0):
            u()
        for a, u in zip(st_units(0), qkv_units(1)):
            a()
            u()
        st1 = st_units(1)
        av0 = av_units(0)
        for idx in range(S_TILES):
            st1[2 * idx]()
            st1[2 * idx + 1]()
            av0[idx]()
        for u in av_units(1, split_last=True):
            u()

    nc.compile()
    return nc


_CACHE = {}


def _get_nc():
    if "nc" not in _CACHE:
        _CACHE["nc"] = _build()
    return _CACHE["nc"]


def _rope_fold(w, bvec):
    """Fold the (position-independent) RoPE rotation into weights/bias.

    Mirrors the reference: inv_freq over arange(0, OUT_DIM, 2)/OUT_DIM,
    angle = 2*S*inv_freq, pairs (2j, 2j+1) rotated by angle_j.
    Computed in float32 to track the reference's f32 arithmetic.
    """
    exps = np.arange(0, OUT_DIM, 2, dtype=np.float32) / np.float32(OUT_DIM)
    inv = (np.float32(1.0) / np.power(np.float32(THETA), exps)).astype(np.float32)
    ang = (np.float32(2.0 * S) * inv).astype(np.float32)
    cos = np.cos(ang).astype(np.float32)
    sin = np.sin(ang).astype(np.float32)

    w2 = w.reshape(OUT_DIM // 2, 2, IN_DIM)
    wr = np.empty_like(w2)
    wr[:, 0] = cos[:, None] * w2[:, 0] - sin[:, None] * w2[:, 1]
    wr[:, 1] = sin[:, None] * w2[:, 0] + cos[:, None] * w2[:, 1]
    b2 = bvec.reshape(OUT_DIM // 2, 2)
    br = np.empty_like(b2)
    br[:, 0] = cos * b2[:, 0] - sin * b2[:, 1]
    br[:, 1] = sin * b2[:, 0] + cos * b2[:, 1]
    return wr.reshape(OUT_DIM, IN_DIM), br.reshape(OUT_DIM)


def _pack_w(w_rot, bf16):
    """[OUT_DIM, IN_DIM] weight -> [128, I_T, OUT_DIM] bf16 SBUF tile layout."""
    wt = np.ascontiguousarray(w_rot.T)                  # [IN_DIM, OUT_DIM]
    return np.ascontiguousarray(
        wt.reshape(I_T, 128, OUT_DIM).transpose(1, 0, 2)
    ).astype(bf16)


def _prep_inputs(x, wq, bq, wk, bk, wv, bv):
    import ml_dtypes
    bf16 = ml_dtypes.bfloat16
    x = np.asarray(x, dtype=np.float32)
    wq_r, bq_r = _rope_fold(np.asarray(wq, np.float32), np.asarray(bq, np.float32))
    wk_r, bk_r = _rope_fold(np.asarray(wk, np.float32), np.asarray(bk, np.float32))
    wv = np.asarray(wv, np.float32)
    bv = np.asarray(bv, np.float32)

    shared = {
        "wq": _pack_w(wq_r, bf16),
        "wk": _pack_w(wk_r, bf16),
        "wv": _pack_w(wv, bf16),
        "bqr": np.ascontiguousarray(bq_r.reshape(O_T, 128).T),
        "bkr": np.ascontiguousarray(bk_r.reshape(O_T, 128).T),
        "bv": bv,
    }
    in_maps = []
    for c in range(N_CORES):
        shard = x[c * B_LOC:(c + 1) * B_LOC]            # [B_LOC, S, IN_DIM]
        # xh[b, h, p, i, s] = shard[b, h*512+s, i*128+p]
        xh = np.ascontiguousarray(
            shard.transpose(0, 2, 1)                     # [b, IN_DIM, S]
            .reshape(B_LOC, I_T, 128, SC, 512)
            .transpose(0, 3, 2, 1, 4)
        ).astype(bf16)
        in_maps.append({"xh": xh, **shared})
    return in_maps


def _execute(in_maps, trace=False, tmpdir=None):
    nc = _get_nc()
    return run_bass_kernel_spmd(
        nc, in_maps, core_ids=list(range(N_CORES)), trace=trace, tmpdir=tmpdir
    )


def kernel(x, wq, bq, wk, bk, wv, bv):
    in_maps = _prep_inputs(x, wq, bq, wk, bk, wv, bv)
    res = _execute(in_maps)
    return np.concatenate(
        [np.asarray(res.results[i]["out"]) for i in range(N_CORES)], axis=0
    ).astype(np.float32)


# revision 23
# speedup vs baseline: 1.0166x; 1.0166x over previous
"""Trainium2 Bass kernel: batch-sharded fused attention (nn_Attention_48893907698232).

Reference computation (per batch b):
    q = x @ wq.T + bq ; k = x @ wk.T + bk ; v = x @ wv.T + bv
    q, k <- fixed RoPE rotation (same rotation at every position)
    out  = softmax((q @ k.T) / sqrt(512)) @ v

Strategy:
  * Data-parallel over batch: 16 batches -> 2 per NeuronCore, 8 cores, no
    collectives (attention is per-batch independent).
  * The RoPE rotation is position-independent here, so it is folded into the
    projection weights/biases on the host (w_rot = R @ w, b_rot = R @ b).
  * Host pre-transposes/pre-tiles x and weights into SBUF tile layout so every
    matmul contraction dim lands on SBUF partitions with zero on-chip
    transposes and every DMA row is contiguous:
      qT[o,s]  = sum_i wqT[i,o] * xT[i,s]     (PSUM out: o on partitions)
      kT[o,s]  likewise
      v[s,o]   = sum_i xT[i,s] * wvT[i,o]     (PSUM out: s on partitions)
      ST[sk,sq]= sum_d kT[d,sk] * qT[d,sq]    -> exp(scale*ST) on ACT -> ET
      out[sq,:]= sum_sk ET[sk,sq] * [v|1]     (ones column => softmax rowsum
                                               lands in output column 256)
      out  <- out[:, :256] * (1 / out[:, 256])
  * Softmax runs without max-subtraction: scores are ~N(0, 0.75) for this
    problem so exp() is far from overflow.
  * Matmul operands are bf16 (full-rate TensorEngine, fp32 PSUM accumulate);
    x and the folded weights are bf16-cast on the host, q/k/v/exp(S) are
    written as bf16 by their producing engines.
  * Input DMAs are split per 128-partition tile and triggered round-robin
    from the sync/gpsimd/vector/scalar queues so the first matmul group's
    operands land as early as possible.
"""

import math
import os
import sys

import numpy as np

os.environ.setdefault("MYCRO_LOCAL_CACHE", "1")
if "/opt/trn_rl_repo" not in sys.path:
    sys.path.insert(0, "/opt/trn_rl_repo")

from contextlib import ExitStack

import concourse.bass as bass
import concourse.tile as tile
from concourse import bacc, mybir
from concourse.bass_utils import run_bass_kernel_spmd

B, S, IN_DIM, OUT_DIM = 16, 1024, 512, 256
THETA = 10000.0
N_CORES = 8
B_LOC = B // N_CORES          # batches per core
I_T = IN_DIM // 128           # 4 contraction tiles for the projections
O_T = OUT_DIM // 128          # 2 feature tiles
S_TILES = S // 128            # 8 sequence tiles
SC = S // 512                 # 2 sequence chunks of 512
SCALE = 1.0 / math.sqrt(IN_DIM)

F32 = mybir.dt.float32
BF16 = mybir.dt.bfloat16


def _build():
    nc = bacc.Bacc(
        "TRN2",
        target_bir_lowering=False,
        debug=False,
        enable_asserts=False,
        num_devices=N_CORES,
    )
    # x pre-tiled on host: xh[b, h, p, i, s] = x[global_batch, h*512+s, i*128+p]
    # (p-major so each DMA partition row is I_T*512*2B = 4KB contiguous)
    xh = nc.dram_tensor(
        "xh", [B_LOC, SC, 128, I_T, 512], BF16, kind="ExternalInput"
    ).ap()
    # weights pre-tiled on host: w[p, i, o] = w_rot.T[i*128+p, o]
    wq = nc.dram_tensor("wq", [128, I_T, OUT_DIM], BF16, kind="ExternalInput").ap()
    wk = nc.dram_tensor("wk", [128, I_T, OUT_DIM], BF16, kind="ExternalInput").ap()
    wv = nc.dram_tensor("wv", [128, I_T, OUT_DIM], BF16, kind="ExternalInput").ap()
    bqr = nc.dram_tensor("bqr", [128, O_T], F32, kind="ExternalInput").ap()
    bkr = nc.dram_tensor("bkr", [128, O_T], F32, kind="ExternalInput").ap()
    bv = nc.dram_tensor("bv", [OUT_DIM], F32, kind="ExternalInput").ap()
    out = nc.dram_tensor("out", [B_LOC, S, OUT_DIM], F32, kind="ExternalOutput").ap()

    with tile.TileContext(nc) as tc, ExitStack() as ctx:
        sb = ctx.enter_context(tc.tile_pool(name="sb", bufs=1))
        small = ctx.enter_context(tc.tile_pool(name="small", bufs=4))
        ps_pj = ctx.enter_context(tc.tile_pool(name="ps_pj", bufs=3, space="PSUM"))
        ps_s = ctx.enter_context(tc.tile_pool(name="ps_s", bufs=3, space="PSUM"))
        ps_av = ctx.enter_context(tc.tile_pool(name="ps_av", bufs=2, space="PSUM"))

        # DMA trigger queues with explicit assignment: the first matmul's two
        # dependencies (x00i0, wq) go on DIFFERENT rings so its latency is
        # max(128KB, 256KB), not their sum. (Tensor queue stays clean.)
        S_, G_, C_ = nc.sync, nc.gpsimd, nc.scalar

        xsb = {}

        def load_xi(eng, b, h, i):
            t = sb.tile([128, 512], BF16, name=f"x{b}h{h}i{i}")
            eng.dma_start(out=t, in_=xh[b, h, :, i, :])
            xsb[b, h, i] = t

        load_xi(S_, 0, 0, 0)
        wq_sb = sb.tile([128, I_T, OUT_DIM], BF16, name="wq_sb")
        G_.dma_start(out=wq_sb, in_=wq)
        load_xi(C_, 0, 0, 1)
        load_xi(S_, 0, 0, 2)
        load_xi(G_, 0, 0, 3)
        wk_sb = sb.tile([128, I_T, OUT_DIM], BF16, name="wk_sb")
        C_.dma_start(out=wk_sb, in_=wk)
        wv_sb = sb.tile([128, I_T, OUT_DIM], BF16, name="wv_sb")
        S_.dma_start(out=wv_sb, in_=wv)
        bqr_sb = sb.tile([128, O_T], F32, name="bqr_sb")
        C_.dma_start(out=bqr_sb, in_=bqr)
        bkr_sb = sb.tile([128, O_T], F32, name="bkr_sb")
        C_.dma_start(out=bkr_sb, in_=bkr)
        bv_bc = sb.tile([128, OUT_DIM], F32, name="bv_bc")
        S_.dma_start(
            out=bv_bc,
            in_=bass.AP(tensor=bv.tensor, offset=bv.offset,
                        ap=[[0, 128], bv.ap[0]]))
        load_xi(G_, 0, 1, 0)
        load_xi(C_, 0, 1, 1)
        load_xi(S_, 0, 1, 2)
        load_xi(G_, 0, 1, 3)
        load_xi(C_, 1, 0, 0)
        load_xi(S_, 1, 0, 1)
        load_xi(G_, 1, 0, 2)
        load_xi(C_, 1, 0, 3)
        load_xi(S_, 1, 1, 0)
        load_xi(G_, 1, 1, 1)
        load_xi(C_, 1, 1, 2)
        load_xi(S_, 1, 1, 3)

        q_sb = [sb.tile([128, O_T, S], BF16, name=f"q{b}") for b in range(B_LOC)]
        k_sb = [sb.tile([128, O_T, S], BF16, name=f"k{b}") for b in range(B_LOC)]
        v_sb = [
            sb.tile([128, S_TILES, OUT_DIM + 1], BF16, name=f"v{b}")
            for b in range(B_LOC)
        ]
        e_sb = [sb.tile([128, S_TILES, S], BF16, name=f"e{b}") for b in range(B_LOC)]

        # ---- work-unit generators (each unit = one PSUM group + epilogue) ----
        def qkv_units(b):
            units = []
            for h in range(SC):
                for w_s, b_s, dst in (
                    (wq_sb, bqr_sb, q_sb[b]),
                    (wk_sb, bkr_sb, k_sb[b]),
                ):
                    for o in range(O_T):
                        def f(h=h, o=o, w_s=w_s, b_s=b_s, dst=dst):
                            ps = ps_pj.tile([128, 512], F32, tag="pj", name="pspj")
                            for i in range(I_T):
                                nc.tensor.matmul(
                                    ps,
                                    w_s[:, i, o * 128:(o + 1) * 128],
                                    xsb[b, h, i],
                                    start=(i == 0),
                                    stop=(i == I_T - 1),
                                )
                            nc.vector.tensor_scalar_add(
                                dst[:, o, h * 512:(h + 1) * 512], ps,
                                b_s[:, o:o + 1],
                            )
                        units.append(f)
                for j in range(S_TILES // SC):
                    def f(h=h, j=j):
                        st = h * (S_TILES // SC) + j
                        ps = ps_pj.tile([128, OUT_DIM], F32, tag="pj", name="pspj")
                        for i in range(I_T):
                            nc.tensor.matmul(
                                ps,
                                xsb[b, h, i][:, j * 128:(j + 1) * 128],
                                wv_sb[:, i, :],
                                start=(i == 0),
                                stop=(i == I_T - 1),
                            )
                        nc.vector.tensor_add(v_sb[b][:, st, 0:OUT_DIM], ps, bv_bc)
                        nc.vector.memset(v_sb[b][:, st, OUT_DIM:OUT_DIM + 1], 1.0)
                    units.append(f)
            return units

        def st_units(b):
            # scores^T tile (sk on partitions, sq chunk on free) + fused exp
            units = []
            for h in range(SC):
                for sk in range(S_TILES):
                    def f(h=h, sk=sk):
                        ps = ps_s.tile([128, 512], F32, tag="s", name="pss")
                        for d in range(O_T):
                            nc.tensor.matmul(
                                ps,
                                k_sb[b][:, d, sk * 128:(sk + 1) * 128],
                                q_sb[b][:, d, h * 512:(h + 1) * 512],
                                start=(d == 0),
                                stop=(d == O_T - 1),
                            )
                        nc.scalar.activation(
                            out=e_sb[b][:, sk, h * 512:(h + 1) * 512],
                            in_=ps,
                            func=mybir.ActivationFunctionType.Exp,
                            scale=SCALE,
                        )
                    units.append(f)
            return units

        def av_units(b, split_last=False):
            units = []
            for sq in range(S_TILES):
                def f(sq=sq):
                    ps = ps_av.tile([128, OUT_DIM + 1], F32, tag="av", name="psav")
                    for sk in range(S_TILES):
                        nc.tensor.matmul(
                            ps,
                            e_sb[b][:, sk, sq * 128:(sq + 1) * 128],
                            v_sb[b][:, sk, :],
                            start=(sk == 0),
                            stop=(sk == S_TILES - 1),
                        )
                    rec = small.tile([128, 1], F32, tag="rec", name="rec")
                    nc.vector.reciprocal(rec, ps[:, OUT_DIM:OUT_DIM + 1])
                    ot = small.tile([128, OUT_DIM], F32, tag="ot", name="ot")
                    if split_last and sq == S_TILES - 1:
                        half = OUT_DIM // 2
                        nc.vector.tensor_scalar_mul(ot[:, 0:half],
                                                    ps[:, 0:half], rec)
                        nc.sync.dma_start(
                            out=out[b, sq * 128:(sq + 1) * 128, 0:half],
                            in_=ot[:, 0:half],
                        )
                        nc.vector.tensor_scalar_mul(ot[:, half:OUT_DIM],
                                                    ps[:, half:OUT_DIM], rec)
                        nc.scalar.dma_start(
                            out=out[b, sq * 128:(sq + 1) * 128, half:OUT_DIM],
                            in_=ot[:, half:OUT_DIM],
                        )
                    else:
                        nc.vector.tensor_scalar_mul(ot, ps[:, 0:OUT_DIM], rec)
                        nc.sync.dma_start(
                            out=out[b, sq * 128:(sq + 1) * 128, :], in_=ot
                        )
                units.append(f)
            return units

        # ---- PE warm-up: keep TensorE busy while input DMAs land so the
        # HAM clock gate reaches 8/8 before the first real matmul ----
        junk = sb.tile([128, 128], BF16, name="junk")
        nc.vector.memset(junk, 0.0)
        for _ in range(8):
            wps = ps_pj.tile([128, 128], F32, tag="pj", name="warmps")
            for _k in range(4):
                nc.tensor.matmul(wps, junk, junk, start=(_k == 0),
                                 stop=(_k == 3))

        # ---- emission: interleave phases so PE never starves ----
        for u in qkv_units(0):
            u()
        for a, u in zip(st_units(0), qkv_units(1)):
            a()
            u()
        st1 = st_units(1)
        av0 = av_units(0)
        for idx in range(S_TILES):
            st1[2 * idx]()
            st1[2 * idx + 1]()
            av0[idx]()
        for u in av_units(1, split_last=True):
            u()

    nc.compile()
    return nc


_CACHE = {}


def _get_nc():
    if "nc" not in _CACHE:
        _CACHE["nc"] = _build()
    return _CACHE["nc"]


def _rope_fold(w, bvec):
    """Fold the (position-independent) RoPE rotation into weights/bias.

    Mirrors the reference: inv_freq over arange(0, OUT_DIM, 2)/OUT_DIM,
    angle = 2*S*inv_freq, pairs (2j, 2j+1) rotated by angle_j.
    Computed in float32 to track the reference's f32 arithmetic.
    """
    exps = np.arange(0, OUT_DIM, 2, dtype=np.float32) / np.float32(OUT_DIM)
    inv = (np.float32(1.0) / np.power(np.float32(THETA), exps)).astype(np.float32)
    ang = (np.float32(2.0 * S) * inv).astype(np.float32)
    cos = np.cos(ang).astype(np.float32)
    sin = np.sin(ang).astype(np.float32)

    w2 = w.reshape(OUT_DIM // 2, 2, IN_DIM)
    wr = np.empty_like(w2)
    wr[:, 0] = cos[:, None] * w2[:, 0] - sin[:, None] * w2[:, 1]
    wr[:, 1] = sin[:, None] * w2[:, 0] + cos[:, None] * w2[:, 1]
    b2 = bvec.reshape(OUT_DIM // 2, 2)
    br = np.empty_like(b2)
    br[:, 0] = cos * b2[:, 0] - sin * b2[:, 1]
    br[:, 1] = sin * b2[:, 0] + cos * b2[:, 1]
    return wr.reshape(OUT_DIM, IN_DIM), br.reshape(OUT_DIM)


def _pack_w(w_rot, bf16):
    """[OUT_DIM, IN_DIM] weight -> [128, I_T, OUT_DIM] bf16 SBUF tile layout."""
    wt = np.ascontiguousarray(w_rot.T)                  # [IN_DIM, OUT_DIM]
    return np.ascontiguousarray(
        wt.reshape(I_T, 128, OUT_DIM).transpose(1, 0, 2)
    ).astype(bf16)


def _prep_inputs(x, wq, bq, wk, bk, wv, bv):
    import ml_dtypes
    bf16 = ml_dtypes.bfloat16
    x = np.asarray(x, dtype=np.float32)
    wq_r, bq_r = _rope_fold(np.asarray(wq, np.float32), np.asarray(bq, np.float32))
    wk_r, bk_r = _rope_fold(np.asarray(wk, np.float32), np.asarray(bk, np.float32))
    wv = np.asarray(wv, np.float32)
    bv = np.asarray(bv, np.float32)

    shared = {
        "wq": _pack_w(wq_r, bf16),
        "wk": _pack_w(wk_r, bf16),
        "wv": _pack_w(wv, bf16),
        "bqr": np.ascontiguousarray(bq_r.reshape(O_T, 128).T),
        "bkr": np.ascontiguousarray(bk_r.reshape(O_T, 128).T),
        "bv": bv,
    }
    in_maps = []
    for c in range(N_CORES):
        shard = x[c * B_LOC:(c + 1) * B_LOC]            # [B_LOC, S, IN_DIM]
        # xh[b, h, p, i, s] = shard[b, h*512+s, i*128+p]
        xh = np.ascontiguousarray(
            shard.transpose(0, 2, 1)                     # [b, IN_DIM, S]
            .reshape(B_LOC, I_T, 128, SC, 512)
            .transpose(0, 3, 2, 1, 4)
        ).astype(bf16)
        in_maps.append({"xh": xh, **shared})
    return in_maps


def _execute(in_maps, trace=False, tmpdir=None):
    nc = _get_nc()
    return run_bass_kernel_spmd(
        nc, in_maps, core_ids=list(range(N_CORES)), trace=trace, tmpdir=tmpdir
    )


def kernel(x, wq, bq, wk, bk, wv, bv):
    in_maps = _prep_inputs(x, wq, bq, wk, bk, wv, bv)
    res = _execute(in_maps)
    return np.concatenate(
        [np.asarray(res.results[i]["out"]) for i in range(N_CORES)], axis=0
    ).astype(np.float32)
